# revision 7
# baseline (speedup 1.0000x reference)
"""MoE gating kernel for Trainium2 (8 NeuronCores, SPMD).

Computes, for x [4, 4096, 2048] f32 and W [64, 2048] f32:
    logits = x_flat @ W.T          # [16384, 64]
    top2 values/indices (sorted descending), softmax over the top-2 logits
Returns (indices int32 [16384, 2], values f32 [16384, 2]) — matching
jax.lax.top_k + softmax in the reference.

Strategy (per core, 2048 tokens = 4 blocks x 512), all fp32 for exact
index agreement with the fp32 reference:
  - DMA x naturally (tokens on partitions, D contiguous) — full HBM BW.
  - PE-transpose 128x128 tiles to put D on partitions.
  - Gating matmul fp32, col-packed: even d-chunks accumulate into PSUM
    partitions 0:64, odd chunks into 64:128 (two concurrent col groups),
    with W.T chunks [128d, 64e] stationary and x.T [128d, 512t] moving.
  - PE-transpose logits back to [128t, 128(2x64e)]; DVE adds the two
    halves; DVE max/max_index for top-2; ACT sigmoid for the 2-way
    softmax; tiny DMA out.
"""

import sys

for _p in ("/opt/trn_rl_repo", "/root/problem/work"):
    if _p not in sys.path:
        sys.path.insert(0, _p)

import numpy as np

import concourse.bass as bass
import concourse.mybir as mybir
from concourse.tile import TileContext
from concourse.bass_utils import run_bass_kernel_spmd

N_CORES = 8
TOKENS = 16384
D = 2048
E = 64
TOK_PER_CORE = TOKENS // N_CORES  # 2048
BLOCK = 512                       # tokens per matmul block
N_BLOCKS = TOK_PER_CORE // BLOCK  # 4
JTILES = BLOCK // 128             # 4
KCHUNKS = D // 128                # 16
NTILES = TOK_PER_CORE // 128      # 16

F32 = mybir.dt.float32
U32 = mybir.dt.uint32

_CACHE = {}


def _split_multi_waits(nc, max_waits=1):
    """walrus in this env supports only ONE sync wait per instruction's
    sync_info; split extras onto preceding NOPs on the same engine."""
    n = 0
    for fn in nc.m.functions:
        for bb in fn.blocks:
            out = []
            for inst in bb.instructions:
                si = inst.sync_info
                if si is not None and si.on_wait is not None and len(si.on_wait) > max_waits:
                    waits = list(si.on_wait)
                    head, tail = waits[:-max_waits], waits[-max_waits:]
                    k = 0
                    while head:
                        chunk, head = head[:max_waits], head[max_waits:]
                        out.append(mybir.InstNoOp(
                            name=f"{inst.name}-wsplit{k}",
                            engine=inst.engine, ins=[], outs=[],
                            sync_info=mybir.SyncInfo(on_wait=chunk, on_update=[]),
                        ))
                        k += 1
                        n += 1
                    inst.sync_info = mybir.SyncInfo(
                        on_wait=tail, on_update=list(si.on_update or []))
                out.append(inst)
            bb.instructions = out
    return n


def build_nc():
    nc = bass.Bass(trn_type="TRN2")
    x = nc.dram_tensor("x", [TOK_PER_CORE, D], F32, kind="ExternalInput")
    w = nc.dram_tensor("W", [E, D], F32, kind="ExternalInput")
    # identity supplied as an input: gpsimd memset/affine_select costs a
    # fixed setup and float32r variants fail this walrus' ISA check
    ident_in = nc.dram_tensor("ident", [128, 128], F32, kind="ExternalInput")
    out_val = nc.dram_tensor("out_val", [128, NTILES, 2], F32, kind="ExternalOutput")
    out_idx = nc.dram_tensor("out_idx", [128, NTILES, 2], U32, kind="ExternalOutput")

    # DRAM view: token = b*512 + j*128 + p  ->  [b, p, j, d]
    x_v = x.rearrange("(b j p) d -> b p j d", b=N_BLOCKS, j=JTILES, p=128)

    with TileContext(nc) as tc:
        with (
            tc.tile_pool(name="singles", bufs=1) as singles,
            tc.tile_pool(name="xb", bufs=2) as xb_pool,
            tc.tile_pool(name="xt", bufs=17) as xt_pool,
            tc.tile_pool(name="lg", bufs=2) as lg_pool,
            tc.tile_pool(name="small", bufs=1) as small,
            tc.tile_pool(name="psT", bufs=3, space="PSUM") as psT_pool,
            tc.tile_pool(name="psL", bufs=2, space="PSUM") as psL_pool,
            tc.tile_pool(name="psS", bufs=2, space="PSUM") as psS_pool,
        ):
            ident = singles.tile([128, 128], F32)
            nc.sync.dma_start(out=ident, in_=ident_in[:, :])

            # W -> SBUF, then 16 PE transposes -> WT [128d, 16c x 64e]
            w_sb = singles.tile([E, D], F32)
            nc.sync.dma_start(out=w_sb, in_=w[:, :])
            wt = singles.tile([128, KCHUNKS * E], F32)
            for c in range(KCHUNKS):
                ps = psS_pool.tile([128, 128], F32, tag="psS")
                nc.tensor.transpose(ps[:, :E], w_sb[:, c * 128:(c + 1) * 128],
                                    ident[:E, :E])
                nc.vector.tensor_copy(wt[:, c * E:(c + 1) * E], ps[:, :E])

            ltok = singles.tile([128, NTILES * E], F32)
            maxb = small.tile([128, NTILES, 8], F32)
            idxb = small.tile([128, NTILES, 8], U32)
            d10 = small.tile([128, NTILES], F32)
            valb = small.tile([128, NTILES, 2], F32)
            idxo = small.tile([128, NTILES, 2], U32)

            for b in range(N_BLOCKS):
                xb = xb_pool.tile([128, JTILES, D], F32, tag="xb")
                if b == 0:
                    # split first block's load so PE starts ~3us in
                    for j in range(JTILES):
                        nc.sync.dma_start(out=xb[:, j, :], in_=x_v[b, :, j, :])
                else:
                    nc.sync.dma_start(out=xb, in_=x_v[b])

                # transpose x block: 16 chunks x 4 jtiles -> xT [128d, 512t]
                xts = []
                for c in range(KCHUNKS):
                    psT = psT_pool.tile([128, BLOCK], F32, tag="psT")
                    for j in range(JTILES):
                        nc.tensor.transpose(
                            psT[:, j * 128:(j + 1) * 128],
                            xb[:, j, c * 128:(c + 1) * 128],
                            ident)
                    xt = xt_pool.tile([128, BLOCK], F32, tag="xt")
                    if c % 2 == 0:
                        nc.vector.tensor_copy(xt, psT)
                    else:
                        nc.scalar.copy(out=xt, in_=psT)
                    xts.append(xt)

                # gating matmuls, col-packed: even c -> partitions 0:64,
                # odd c -> 64:128; two col groups run concurrently
                psL = psL_pool.tile([128, BLOCK], F32, tag="psL")
                for c in range(KCHUNKS):
                    g = c % 2
                    nc.tensor.matmul(
                        psL[g * E:(g + 1) * E, :],
                        lhsT=wt[:, c * E:(c + 1) * E], rhs=xts[c],
                        start=(c < 2), stop=(c >= KCHUNKS - 2))

                lgs = lg_pool.tile([128, BLOCK], F32, tag="lg")
                nc.scalar.copy(out=lgs, in_=psL)

                # transpose logits; add the even/odd half-sums (DVE may
                # read at most one PSUM operand, so stage one half in SBUF)
                for k in range(JTILES):
                    t = b * JTILES + k
                    ps2 = psS_pool.tile([128, 128], F32, tag="psS")
                    nc.tensor.transpose(ps2, lgs[:, k * 128:(k + 1) * 128],
                                        ident)
                    lh = lg_pool.tile([128, E], F32, tag="lh")
                    nc.scalar.copy(out=lh, in_=ps2[:, E:128])
                    nc.vector.tensor_add(ltok[:, t * E:(t + 1) * E],
                                         ps2[:, 0:E], lh)

                # top-2 + 2-way softmax for this block's 4 token tiles
                for k in range(JTILES):
                    t = b * JTILES + k
                    nc.vector.max(out=maxb[:, t, :],
                                  in_=ltok[:, t * E:(t + 1) * E])
                    nc.vector.max_index(out=idxb[:, t, :], in_max=maxb[:, t, :],
                                        in_values=ltok[:, t * E:(t + 1) * E])
                s = slice(b * JTILES, (b + 1) * JTILES)
                nc.vector.tensor_sub(d10[:, s], maxb[:, s, 1], maxb[:, s, 0])
                nc.scalar.activation(valb[:, s, 1], d10[:, s],
                                     mybir.ActivationFunctionType.Sigmoid)
                nc.scalar.activation(valb[:, s, 0], d10[:, s],
                                     mybir.ActivationFunctionType.Sigmoid,
                                     scale=-1.0)
                nc.vector.tensor_copy(idxo[:, s, :], idxb[:, s, 0:2])

            nc.sync.dma_start(out=out_val[:, :, :], in_=valb)
            nc.sync.dma_start(out=out_idx[:, :, :], in_=idxo)

    _split_multi_waits(nc)
    return nc


def _get_nc():
    if "nc" not in _CACHE:
        _CACHE["nc"] = build_nc()
    return _CACHE["nc"]


def kernel(x: np.ndarray, W: np.ndarray, _trace=False, _tmpdir=None):
    nc = _get_nc()
    x_flat = np.ascontiguousarray(x.reshape(TOKENS, D).astype(np.float32))
    Wc = np.ascontiguousarray(W.astype(np.float32))
    ident = np.eye(128, dtype=np.float32)
    in_maps = [
        {"x": x_flat[c * TOK_PER_CORE:(c + 1) * TOK_PER_CORE], "W": Wc,
         "ident": ident}
        for c in range(N_CORES)
    ]
    res = run_bass_kernel_spmd(nc, in_maps, core_ids=list(range(N_CORES)),
                               trace=_trace, tmpdir=_tmpdir)
    _CACHE["last_result"] = res
    idx_parts, val_parts = [], []
    for c in range(N_CORES):
        r = res.results[c]
        # [128p, 16t, 2] -> token local = t*128 + p
        val_parts.append(r["out_val"].transpose(1, 0, 2).reshape(TOK_PER_CORE, 2))
        idx_parts.append(r["out_idx"].transpose(1, 0, 2).reshape(TOK_PER_CORE, 2)
                         .astype(np.int32))
    return (np.concatenate(idx_parts, 0), np.concatenate(val_parts, 0))
